# revision 5
# baseline (speedup 1.0000x reference)
import sys
sys.path.insert(0, "/opt/trn_rl_repo")

import numpy as np
import ml_dtypes
from contextlib import ExitStack

import concourse.bass as bass
import concourse.mybir as mybir
import concourse.tile as tile
from concourse import bacc
from concourse.bass_utils import run_bass_kernel_spmd

# ---- problem constants (hardcoded per spec) ----
H, W = 200, 100
NQ, NV, EMB, HEADS, NPT, DH = 2000, 20000, 256, 8, 4, 32
N_CORES = 8
YP = 203                    # padded rows y' = y+1, y' in [0, 202]
NCELL = YP * W              # 20300
NCELL_PAD = 20352           # 159 * 128
NTILE_C = NCELL_PAD // 128  # 159
VW = 20480                  # vt width: cell c + shift 101 + 127 <= 20452
QT = 2048                   # padded queries (16 tiles of 128)
NQT = 16
NQUEUES = 4
F32 = mybir.dt.float32
BF16 = mybir.dt.bfloat16
FP8 = mybir.dt.float8e4
I16 = mybir.dt.int16

_CACHE = {}


def build_kernel(debug=False):
    nc = bacc.Bacc("TRN2", target_bir_lowering=False, debug=False,
                   num_devices=N_CORES, num_swdge_queues=NQUEUES)
    A = mybir.AluOpType
    ACT = mybir.ActivationFunctionType
    dt = nc.dram_tensor
    qT_in = dt("qT", [2, 128, NQ], F32, kind="ExternalInput")
    vT_in = dt("vT", [2, 128, NV], BF16, kind="ExternalInput")
    ref_in = dt("ref", [NQ, 2], F32, kind="ExternalInput")
    wv_in = dt("wv", [2, 128, EMB], BF16, kind="ExternalInput")
    woa_in = dt("woa", [2, 128, 96], F32, kind="ExternalInput")
    boa_in = dt("boa", [128, 96], F32, kind="ExternalInput")
    wout_in = dt("wout", [2, 128, 256], BF16, kind="ExternalInput")
    bout_in = dt("bout", [128, 2], F32, kind="ExternalInput")
    idf_in = dt("idf", [128, 128], F32, kind="ExternalInput")
    idb_in = dt("idb", [128, 128], BF16, kind="ExternalInput")
    outT = dt("outT", [2, 128, NQ], F32, kind="ExternalOutput")
    if debug:
        dbg_oa = dt("dbg_oa", [128, 16 * 96], F32, kind="ExternalOutput")
        dbg_wc = dt("dbg_wc", [128, 16 * 128], F32, kind="ExternalOutput")
        dbg_idx = dt("dbg_idx", [32, QT], I16, kind="ExternalOutput")
        dbg_vp2 = [dt(f"dbg_vp2_{g}", [NCELL_PAD * 512], FP8,
                      kind="ExternalOutput") for g in range(2)]
        dbg_g = dt("dbg_g", [128, NQT * 256], FP8, kind="ExternalOutput")

    with tile.TileContext(nc) as tc, ExitStack() as octx:
        const = octx.enter_context(tc.tile_pool(name="const", bufs=1))
        dram = octx.enter_context(tc.tile_pool(name="dram", bufs=1,
                                               space="DRAM"))
        # two head-group planes: per cell 4h x (y0,y1) x 32d fp8 = 256B;
        # the x+1 corners come from the next cell via a 512B gather elem
        vp2 = [dram.tile([NCELL_PAD * 256], FP8, tag=f"vp2_{g}",
                         name=f"vp2_{g}") for g in range(2)]
        idxd = dram.tile([32 * QT], I16, tag="idxd")

        idf = const.tile([128, 128], F32, tag="idf")
        nc.sync.dma_start(out=idf, in_=idf_in[:, :])
        idb = const.tile([128, 128], BF16, tag="idb")
        nc.sync.dma_start(out=idb, in_=idb_in[:, :])
        wv = [const.tile([128, EMB], BF16, tag=f"wv{k}", name=f"wv{k}")
              for k in range(2)]
        woa = [const.tile([128, 96], F32, tag=f"woa{k}", name=f"woa{k}")
               for k in range(2)]
        wout = [const.tile([128, 256], BF16, tag=f"wo{k}", name=f"wo{k}")
                for k in range(2)]
        for k in range(2):
            nc.sync.dma_start(out=wv[k], in_=wv_in[k])
            nc.sync.dma_start(out=woa[k], in_=woa_in[k])
            nc.sync.dma_start(out=wout[k], in_=wout_in[k])
        bias_oa = const.tile([128, 96], F32, tag="boa")
        nc.sync.dma_start(out=bias_oa, in_=boa_in[:, :])
        bout_sb = const.tile([128, 2], F32, tag="bout")
        nc.sync.dma_start(out=bout_sb, in_=bout_in[:, :])

        persist = octx.enter_context(tc.tile_pool(name="persist", bufs=1))
        qTf = [persist.tile([128, QT], F32, tag=f"qTf{c}", name=f"qTf{c}")
               for c in range(2)]
        wc = persist.tile([128, NQT, 128], F32, tag="wc")
        cttQ = persist.tile([128, NQT, 256], BF16, tag="cttQ")

        for c in range(2):
            nc.vector.memset(qTf[c][:, NQ:QT], 0.0)
            nc.sync.dma_start(out=qTf[c][:, 0:NQ], in_=qT_in[c])

        # ---------- value load (early: on the DMA critical path) ----
        vstack = ExitStack()
        vtp = vstack.enter_context(tc.tile_pool(name="vtp", bufs=1))
        vt = [vtp.tile([128, VW], BF16, tag=f"vt{k}", name=f"vt{k}")
              for k in range(2)]
        for k in range(2):
            nc.vector.memset(vt[k][:, 0:100], 0.0)
            nc.vector.memset(vt[k][:, 100 + NV:VW], 0.0)
            for j in range(4):
                nc.sync.dma_start(
                    out=vt[k][:, 100 + j * 5000:100 + (j + 1) * 5000],
                    in_=vT_in[k][:, j * 5000:(j + 1) * 5000])

        # shifts: y0, y1 rows; x-corners come from cell adjacency
        SHIFTS = [0, 100]
        MXG = 4   # cell-tiles per output DMA

        s2 = vstack.enter_context(tc.tile_pool(name="s2", bufs=2))
        s2p = vstack.enter_context(tc.tile_pool(name="s2p", bufs=3,
                                               space="PSUM"))
        s5ctx = {}

        def emit_proj(hg):
            # quad projection for one head group -> vp2[hg]; pack on ACT
            for cg in range(NTILE_C // MXG + 1):
                tiles = range(cg * MXG, min((cg + 1) * MXG, NTILE_C))
                if not tiles:
                    continue
                mx = s2.tile([128, MXG, 256], FP8, tag="mx")
                for j, ci in enumerate(tiles):
                    c0 = ci * 128
                    ps = s2p.tile([128, 2, 128], F32, tag="pj_ps")
                    for si, sh in enumerate(SHIFTS):
                        for k in range(2):
                            nc.tensor.matmul(
                                ps[:, si, :],
                                vt[k][:, c0 + sh:c0 + sh + 128],
                                wv[k][:, hg * 128:(hg + 1) * 128],
                                start=(k == 0), stop=(k == 1))
                    src_v = ps.rearrange("p c (h d) -> p h c d", h=4)
                    dst_v = mx[:, j, :].rearrange(
                        "p (h c d) -> p h c d", h=4, c=2)
                    if hg == 0 and ci % 2 == 0:
                        nc.vector.tensor_copy(dst_v, src_v)
                    else:
                        nc.scalar.activation(dst_v, src_v, ACT.Copy)
                n = len(tiles)
                nc.sync.dma_start(
                    out=bass.AP(
                        tensor=vp2[hg][:].tensor,
                        offset=vp2[hg][:].offset + cg * MXG * 32768,
                        ap=[[256, 128], [32768, n], [1, 256]]),
                    in_=mx[:, 0:n, :])

        def emit_gathers(hg, ibs):
            s5, s5w, s5u = s5ctx["s5"], s5ctx["s5w"], s5ctx["s5u"]
            for h in range(hg * 4, hg * 4 + 4):
                wt = [None] * 4
                u1 = u2 = None
                for p in range(NPT):
                    row = h * 4 + p
                    g = s5.tile([128, NQT, 512], FP8, tag="g")
                    nc.gpsimd.dma_gather(
                        g,
                        bass.AP(tensor=vp2[hg][:].tensor,
                                offset=vp2[hg][:].offset + (h % 4) * 64,
                                ap=[[256, NCELL_PAD - 4], [1, 512]]),
                        ibs[row], QT, QT, 512, elem_step=256,
                        transpose=False, single_packet=False,
                        queue_num=row % NQUEUES)
                    # x0 ypair at elem bytes 0:64, x1 ypair at 256:320
                    gq01 = g[:, :, 0:64].rearrange(
                        "p t (c d) -> p t c d", c=2)
                    gq23 = g[:, :, 256:320].rearrange(
                        "p t (c d) -> p t c d", c=2)
                    wq = wc.rearrange("p t (c j) -> p t c j", j=4)[
                        :, :, row, :]
                    wtp = s5w.tile([128, NQT, 2, 32], BF16, tag="wt")
                    nc.vector.tensor_mul(
                        wtp, gq01,
                        wq[:, :, 0:2, None].broadcast_to(
                            [128, NQT, 2, 32]))
                    wtq = s5w.tile([128, NQT, 2, 32], BF16, tag="wt2")
                    nc.vector.tensor_mul(
                        wtq, gq23,
                        wq[:, :, 2:4, None].broadcast_to(
                            [128, NQT, 2, 32]))
                    up = s5w.tile([128, NQT, 2, 32], BF16, tag="up")
                    nc.vector.tensor_add(up, wtp, wtq)
                    wt[p] = up
                    if p == 1:
                        u1 = s5u.tile([128, NQT, 2, 32], BF16, tag="u1")
                        nc.vector.tensor_add(u1, wt[0], wt[1])
                    elif p == 3:
                        u2 = s5u.tile([128, NQT, 2, 32], BF16, tag="u2")
                        nc.vector.tensor_add(u2, wt[2], wt[3])
                acc = cttQ[:, :, h * 32:(h + 1) * 32]
                v4 = s5u.tile([128, NQT, 2, 32], BF16, tag="v4")
                nc.vector.tensor_add(v4, u1, u2)
                nc.vector.tensor_add(acc, v4[:, :, 0, :], v4[:, :, 1, :])

        # ---------- head group 0: projection ----------
        emit_proj(0)

        # ---------- stage 3: off/attn projections (f32 for exactness) ----
        with tc.tile_pool(name="s3", bufs=1) as s3, \
             tc.tile_pool(name="s3p", bufs=2, space="PSUM") as s3p:
            oa_sb = s3.tile([128, NQT, 96], F32, tag="oa")
            for t in range(NQT):
                ps = s3p.tile([128, 96], F32, tag="oa_ps")
                for k in range(2):
                    nc.tensor.matmul(ps, qTf[k][:, t * 128:(t + 1) * 128],
                                     woa[k], start=(k == 0), stop=(k == 1))
                nc.vector.tensor_add(oa_sb[:, t, :], ps, bias_oa)

            # ---------- stage 4: coords, weights, indices ----------
            with tc.tile_pool(name="s4", bufs=1) as s4, \
                 tc.tile_pool(name="s4p", bufs=2, space="PSUM") as s4p:
                shp = [128, NQT, 32]
                ref_sb = s4.tile([128, NQT, 2], F32, tag="ref")
                nc.vector.memset(ref_sb, 0.0)
                nc.sync.dma_start(
                    out=ref_sb[:, 0:15, :],
                    in_=ref_in[0:1920, :].rearrange("(t p) c -> p t c",
                                                    p=128))
                nc.sync.dma_start(out=ref_sb[:NQ - 1920, 15, :],
                                  in_=ref_in[1920:NQ, :])
                oav = oa_sb.rearrange("p t (c two) -> p t c two", two=2)
                ox = oav[:, :, 0:32, 0]
                oy = oav[:, :, 0:32, 1]
                awl = oa_sb[:, :, 64:96]

                awe = s4.tile(shp, F32, tag="awe")
                nc.scalar.activation(awe, awl, ACT.Exp)
                s1t = s4.tile([128, NQT, 16], F32, tag="s1t")
                av = awe.rearrange("p t (c two) -> p t c two", two=2)
                nc.vector.tensor_add(s1t, av[:, :, :, 0], av[:, :, :, 1])
                s2t = s4.tile([128, NQT, 8], F32, tag="s2t")
                sv = s1t.rearrange("p t (c two) -> p t c two", two=2)
                nc.vector.tensor_add(s2t, sv[:, :, :, 0], sv[:, :, :, 1])
                rec = s4.tile([128, NQT, 8], F32, tag="rec")
                nc.vector.reciprocal(rec, s2t)
                awn = s4.tile(shp, F32, tag="awn")
                nc.vector.tensor_mul(
                    awn.rearrange("p t (c f) -> p t c f", f=4),
                    awe.rearrange("p t (c f) -> p t c f", f=4),
                    rec[:, :, :, None].broadcast_to([128, NQT, 8, 4]))

                refw = s4.tile([128, NQT, 2], F32, tag="refw")
                nc.vector.tensor_scalar(out=refw[:, :, 0:1],
                                        in0=ref_sb[:, :, 0:1],
                                        scalar1=float(W), scalar2=0.5,
                                        op0=A.mult, op1=A.add)
                nc.vector.tensor_scalar(out=refw[:, :, 1:2],
                                        in0=ref_sb[:, :, 1:2],
                                        scalar1=float(H), scalar2=0.5,
                                        op0=A.mult, op1=A.add)
                px = s4.tile(shp, F32, tag="px")
                nc.vector.tensor_add(px, ox,
                                     refw[:, :, 0:1].broadcast_to(shp))
                py = s4.tile(shp, F32, tag="py")
                nc.vector.tensor_add(py, oy,
                                     refw[:, :, 1:2].broadcast_to(shp))
                nc.vector.tensor_scalar(out=px, in0=px, scalar1=0.0,
                                        scalar2=float(W + 1),
                                        op0=A.max, op1=A.min)
                nc.vector.tensor_scalar(out=py, in0=py, scalar1=0.0,
                                        scalar2=float(YP - 2),
                                        op0=A.max, op1=A.min)
                M23 = float(1 << 23)
                # exact floor: round-to-nearest via +M23, then subtract
                # (rounded > x) to fix the round-up cases
                x0 = s4.tile(shp, F32, tag="x0")
                nc.vector.tensor_scalar(out=x0, in0=px, scalar1=M23,
                                        scalar2=M23, op0=A.add,
                                        op1=A.subtract)
                gtx = s4.tile(shp, F32, tag="gtx")
                nc.vector.tensor_tensor(out=gtx, in0=x0, in1=px,
                                        op=A.is_gt)
                nc.vector.tensor_sub(x0, x0, gtx)
                y0 = s4.tile(shp, F32, tag="y0")
                nc.vector.tensor_scalar(out=y0, in0=py, scalar1=M23,
                                        scalar2=M23, op0=A.add,
                                        op1=A.subtract)
                gty = s4.tile(shp, F32, tag="gty")
                nc.vector.tensor_tensor(out=gty, in0=y0, in1=py,
                                        op=A.is_gt)
                nc.vector.tensor_sub(y0, y0, gty)
                fx = s4.tile(shp, F32, tag="fx")
                nc.vector.tensor_sub(fx, px, x0)
                fy = s4.tile(shp, F32, tag="fy")
                nc.vector.tensor_sub(fy, py, y0)

                idxf = s4.tile(shp, F32, tag="idxf")
                cellf = s4.tile(shp, F32, tag="cellf")
                nc.vector.scalar_tensor_tensor(out=cellf, in0=y0,
                                               scalar=float(W), in1=x0,
                                               op0=A.mult, op1=A.add)
                nc.vector.tensor_scalar(out=idxf, in0=cellf,
                                        scalar1=1.0, scalar2=0.0,
                                        op0=A.subtract, op1=A.max)
                nc.vector.tensor_scalar(out=idxf, in0=idxf,
                                        scalar1=float(NCELL - 1),
                                        scalar2=None, op0=A.min)

                ga1 = s4.tile(shp, F32, tag="ga1")
                nc.vector.tensor_scalar(out=ga1, in0=x0, scalar1=0.5,
                                        scalar2=None, op0=A.is_ge)
                ga2 = s4.tile(shp, F32, tag="ga2")
                nc.vector.tensor_scalar(out=ga2, in0=x0,
                                        scalar1=float(W) + 0.5,
                                        scalar2=None, op0=A.is_le)
                gb = s4.tile(shp, F32, tag="gb")
                nc.vector.tensor_scalar(out=gb, in0=x0,
                                        scalar1=float(W) - 0.5,
                                        scalar2=None, op0=A.is_le)
                fx1 = s4.tile(shp, F32, tag="fx1")
                nc.vector.tensor_scalar(out=fx1, in0=fx, scalar1=-1.0,
                                        scalar2=1.0, op0=A.mult, op1=A.add)
                fy1 = s4.tile(shp, F32, tag="fy1")
                nc.vector.tensor_scalar(out=fy1, in0=fy, scalar1=-1.0,
                                        scalar2=1.0, op0=A.mult, op1=A.add)
                aa = s4.tile(shp, F32, tag="aa")
                nc.vector.tensor_mul(aa, fx1, ga1)
                nc.vector.tensor_mul(aa, aa, ga2)
                nc.vector.tensor_mul(aa, aa, awn)
                bb = s4.tile(shp, F32, tag="bb")
                nc.vector.tensor_mul(bb, fx, gb)
                nc.vector.tensor_mul(bb, bb, awn)

                # corner order: (x0y0, x0y1, x1y0, x1y1)
                wcv = wc.rearrange("p t (c j) -> p t c j", j=4)
                nc.vector.tensor_mul(wcv[:, :, :, 0], aa, fy1)
                nc.vector.tensor_mul(wcv[:, :, :, 1], aa, fy)
                nc.vector.tensor_mul(wcv[:, :, :, 2], bb, fy1)
                nc.vector.tensor_mul(wcv[:, :, :, 3], bb, fy)

                # gather reads its idx table round-robin across the 16
                # wrap partitions: output elem i <- partition i%16, slot
                # i//16. DRAM col (p*128 + j) must hold idx of query
                # j*16 + p.
                idxT = s4.tile([32, QT], F32, tag="idxT")
                for t in range(NQT):
                    ps2 = s4p.tile([128, 128], F32, tag="tr2_ps")
                    nc.tensor.transpose(ps2[:32, :], idxf[:, t, :], idf)
                    dstv = idxT.rearrange("r (p j) -> r p j",
                                          p=16)[:, :, t * 8:(t + 1) * 8]
                    srcv = ps2[:32, :].rearrange("r (j p) -> r p j", p=16)
                    nc.vector.tensor_copy(dstv, srcv)
                idx16 = s4.tile([32, QT], I16, tag="idx16")
                nc.vector.tensor_copy(idx16, idxT)
                nc.sync.dma_start(
                    out=idxd[:].rearrange("(p f) -> p f", p=32), in_=idx16)
                if debug:
                    nc.sync.dma_start(out=dbg_idx[:, :], in_=idx16)
                    nc.sync.dma_start(
                        out=dbg_oa[:, :],
                        in_=oa_sb.rearrange("p t c -> p (t c)"))
                    nc.sync.dma_start(
                        out=dbg_wc[:, :],
                        in_=wc.rearrange("p t c -> p (t c)"))


        # gather pools open only after the coords temps are freed
        gstack = ExitStack()
        s5ctx["s5"] = gstack.enter_context(tc.tile_pool(name="s5", bufs=4))
        s5ctx["s5w"] = gstack.enter_context(tc.tile_pool(name="s5w", bufs=3))
        s5ctx["s5u"] = gstack.enter_context(tc.tile_pool(name="s5u", bufs=2))
        ibp = gstack.enter_context(tc.tile_pool(name="ibp", bufs=1))

        # ---------- early ib loads (all rows; Sync queue before hg1) ----
        ibs = []
        for row in range(32):
            ib = ibp.tile([128, 128], I16, tag=f"ib{row}", name=f"ib{row}")
            nc.scalar.dma_start(out=ib, in_=bass.AP(
                tensor=idxd[:].tensor,
                offset=idxd[:].offset + row * QT,
                ap=[[0, 8], [128, 16], [1, 128]]))
            ibs.append(ib)

        # ---------- hg0 gathers, hg1 projection (overlapped) ----------
        emit_gathers(0, ibs)
        emit_proj(1)
        if debug:
            for gi in range(2):
                nc.sync.dma_start(out=dbg_vp2[gi][:], in_=vp2[gi][:])
        emit_gathers(1, ibs)
        gstack.close()
        vstack.close()

        # ---------- stage 6: transpose + output projection ----------
        with tc.tile_pool(name="s6", bufs=2) as s6, \
             tc.tile_pool(name="s6c", bufs=1) as s6c, \
             tc.tile_pool(name="s6p", bufs=2, space="PSUM") as s6p:
            ctt = [s6c.tile([128, QT], BF16, tag=f"ct{c}", name=f"ct{c}")
                   for c in range(2)]
            for t in range(NQT):
                pst = s6p.tile([128, 256], BF16, tag="ct_ps")
                for ch in range(2):
                    nc.tensor.transpose(
                        pst[:, ch * 128:(ch + 1) * 128],
                        cttQ[:, t, ch * 128:(ch + 1) * 128], idb)
                for ch in range(2):
                    if t % 2 == 0:
                        nc.vector.tensor_copy(
                            ctt[ch][:, t * 128:(t + 1) * 128],
                            pst[:, ch * 128:(ch + 1) * 128])
                    else:
                        nc.scalar.activation(
                            ctt[ch][:, t * 128:(t + 1) * 128],
                            pst[:, ch * 128:(ch + 1) * 128], ACT.Copy)
            for oh in range(2):
                for qc in range(4):
                    q0 = qc * 512
                    cols = min(NQ, q0 + 512) - q0
                    ps = s6p.tile([128, 512], F32, tag="out_ps")
                    for ch in range(2):
                        nc.tensor.matmul(
                            ps,
                            wout[ch][:, oh * 128:(oh + 1) * 128],
                            ctt[ch][:, q0:q0 + 512],
                            start=(ch == 0), stop=(ch == 1))
                    ot = s6.tile([128, 512], F32, tag="ot")
                    nc.vector.tensor_scalar(out=ot[:, 0:cols],
                                            in0=ps[:, 0:cols],
                                            scalar1=bout_sb[:, oh:oh + 1],
                                            scalar2=None, op0=A.add)
                    nc.vector.tensor_add(ot[:, 0:cols], ot[:, 0:cols],
                                         qTf[oh][:, q0:q0 + cols])
                    nc.sync.dma_start(out=outT[oh][:, q0:q0 + cols],
                                      in_=ot[:, 0:cols])

    nc.finalize()
    return nc


def _prep_shared(inputs):
    bf = ml_dtypes.bfloat16
    W_val = np.asarray(inputs["W_val"], np.float32)
    W_off = np.asarray(inputs["W_off"], np.float32)
    W_attn = np.asarray(inputs["W_attn"], np.float32)
    W_out = np.asarray(inputs["W_out"], np.float32)
    b_off = np.asarray(inputs["b_off"], np.float32)
    b_attn = np.asarray(inputs["b_attn"], np.float32)
    b_val = np.asarray(inputs["b_val"], np.float32)
    b_out = np.asarray(inputs["b_out"], np.float32)
    assert np.allclose(b_val, 0.0), "kernel assumes b_val == 0"
    woa = np.concatenate([W_off, W_attn], axis=1)
    boa = np.concatenate([b_off, b_attn], axis=0)
    idf = np.eye(128, dtype=np.float32)
    return dict(
        wv=np.ascontiguousarray(W_val.reshape(2, 128, 256)).astype(bf),
        woa=np.ascontiguousarray(woa.reshape(2, 128, 96)),
        boa=np.ascontiguousarray(np.broadcast_to(boa, (128, 96))),
        wout=np.ascontiguousarray(W_out.reshape(2, 128, 256)).astype(bf),
        bout=np.ascontiguousarray(b_out.reshape(2, 128).T),
        idf=idf, idb=idf.astype(bf))


def make_in_maps(inputs):
    bf = ml_dtypes.bfloat16
    shared = _prep_shared(inputs)
    q = np.asarray(inputs["query"], np.float32)
    v = np.asarray(inputs["value"], np.float32)
    ref = np.asarray(inputs["reference_points"], np.float32)
    in_maps = []
    for c in range(N_CORES):
        qT = np.ascontiguousarray(q[:, c, :].T).reshape(2, 128, NQ)
        vT = np.ascontiguousarray(v[:, c, :].T.astype(bf)).reshape(
            2, 128, NV)
        in_maps.append(dict(
            qT=qT, vT=vT,
            ref=np.ascontiguousarray(ref[c, :, 0, :]),
            **shared))
    return in_maps


def kernel(**inputs):
    if "nc" not in _CACHE:
        _CACHE["nc"] = build_kernel(debug=False)
    nc = _CACHE["nc"]
    in_maps = make_in_maps(inputs)
    res = run_bass_kernel_spmd(nc, in_maps, core_ids=list(range(N_CORES)))
    out = np.empty((NQ, N_CORES, EMB), np.float32)
    for c in range(N_CORES):
        oT = res.results[c]["outT"]
        out[:, c, :] = oT.reshape(256, NQ).T
    return out
